# revision 36
# baseline (speedup 1.0000x reference)
"""CBOW (nn_CBOW_88991722373900) Trainium2 kernel.

Full-input contract: kernel(context_words[10,128000] f32, W_in[300,128000] f32,
W_out[128000,300] f32) -> softmax probabilities [128000] f32.

Strategy (8-way tensor/model parallel over the vocab dim V):
  - shard V into 8 chunks of 16000; each core holds its slice of both weight
    matrices. W_in/ctx/PE-half W_out in fp8-e4m3 (host pre-scales W_in x128,
    W_out x16, hidden x8, compensated exactly in the exp scale); the
    DVE-half W_out stays bf16 (DVE has no fp8 packing).
  - DMA order: w1 groups first on both HWDGE rings, then w2, so GEMM1
    finishes and triggers the hidden AllGather before the runtime's CC
    barrier completes (the barrier starts ~21us into execution on its own
    and gates the first collective; GEMM1 speed is off the critical path).
  - GEMM1: partial hidden[10,300] = ctx chunks (stationary) x w1 tiles
    (moving), accumulated in PSUM over 125 v-chunks (fp8 matmuls).
  - C-reduce (ones-matmul) -> AllGather(600B bf16) -> rank-sum.
  - GEMM2 split across two engines (post-AllGather critical path):
      PE half  (v-blocks 0..PEB-1): W_out col-blocks stationary (fp8, FWL),
               hidden col (fp8 x8) moving -> PSUM [128, PEB], logits x128
      DVE half: fused tensor_tensor_reduce of bf16 w2r against the
               partition-broadcast bf16 hidden, one op per v-block
    v mapped so partition p holds contiguous v = 125*p + b
  - softmax: exp on ScalarE with scale folding the fp8 pre-scales,
    AllGather(4B) of the local denominators, then a rank-1 ones-matmul
    broadcasts the reciprocal to all partitions (no broadcast-read DMA).
  - all small/control DMAs ride the HWDGE rings (0.6us fixed cost vs
    ~2us on the SWDGE/gpsimd path).
"""

import numpy as np
import ml_dtypes

import concourse.bass as bass
import concourse.mybir as mybir
from concourse import tile
from concourse.bass_utils import run_bass_kernel_spmd
from concourse.vector_clock import ScopedClock, VectorClock

V = 128000
N = 300
C = 10
W = 8              # cores
VL = V // W        # 16000 vocab per core
NJ = VL // 128     # 125 v-chunks for GEMM1
NB = VL // 128     # 125 v-blocks for GEMM2
NCH = [(0, 128), (128, 128), (256, 44)]  # n-chunks
PEB = 70           # v-blocks on the PE half of GEMM2
DVB = NB - PEB     # v-blocks on the DVE half
W2G = 18           # PE-half v-blocks per w2 SBUF tile group
NG2 = (PEB + W2G - 1) // W2G
DVG = 14           # DVE-half v-blocks per w2r SBUF tile group
NGR = (DVB + DVG - 1) // DVG
# PE half covers n in [0, 256) (2 LDW+MM pairs per block instead of 3);
# the n in [256, 300) tail of those blocks runs on the DVE (w2p44).
NPE = 256
NTL = N - NPE      # 44
NCH_PE = [(0, 128), (128, 128)]

# host-side power-of-2 pre-scales (compensated exactly on-chip)
S_W1 = 128.0       # W_in scale into fp8 range
S_W2 = 16.0        # W_out scale into fp8 range (PE half only)
S_H = 8.0          # hidden scale into fp8 range (PE half)

BF16 = mybir.dt.bfloat16
F8 = mybir.dt.float8e4
F32 = mybir.dt.float32
NP_BF16 = ml_dtypes.bfloat16
NP_F8 = ml_dtypes.float8_e4m3


def _patched_drain_and_barrier(self, tick_clock, wait_clock):
    """Tail-drain waits split into 1-wait NOPs: this walrus build's CTRL
    instructions only encode a single sync wait."""
    vc = tick_clock.global_clock
    procs = [(p, vc[p]) for p in range(len(vc)) if vc[p] > 0]
    for i, (p, t) in enumerate(procs):
        pvc = VectorClock([0] * len(vc))
        pvc.require_at_least(p, t)
        nop_inst = self.nc.sync.nop(nofuse=True, hint=f"tail_wait_{i}")
        wait_clock.add_sem_waits(nop_inst.ins, ScopedClock({None: pvc}))
    self.nc.sync.drain()
    self.nc.all_engine_barrier(sem_only=True)
    assert self.sems is not None
    popped = self.nc._tile_sem_poison_stack.pop()
    assert popped is self._sem_poison
    self.nc.clear_and_free_semaphores(list(self.sems.allocated().values()))
    self.nc.all_engine_barrier(sem_only=True)


tile.TileContext._drain_and_barrier = _patched_drain_and_barrier


def _split_multi_waits(nc):
    """This walrus build encodes at most ONE sync wait per instruction. Hoist
    excess waits onto same-engine NoOps inserted immediately before."""
    import bass_rust

    ctr = [0]

    def make_nop(engine, wait):
        ctr[0] += 1
        nop = mybir.InstNoOp(name=f"I-wsplit{ctr[0]}", engine=engine)
        nop.bass_nofuse = True
        nop.sync_info = bass_rust.SyncInfo(on_wait=[wait], on_update=[])
        nc.register_instruction(nop, overwrite=True)
        return nop

    for bb in nc.main_func.blocks:
        out = []
        for ins in bb.instructions:
            si = ins.sync_info
            if si is not None and si.on_wait and len(si.on_wait) > 1:
                waits = list(si.on_wait)
                for w in waits[:-1]:
                    out.append(make_nop(ins.engine, w))
                ins.sync_info = bass_rust.SyncInfo(
                    on_wait=[waits[-1]], on_update=list(si.on_update)
                )
            out.append(ins)
        bb.instructions = out


def build_kernel():
    nc = bass.Bass()

    ctxp = nc.dram_tensor("ctxp", [128, NJ * C], F8, kind="ExternalInput")
    # w1t packed partition-major on host: w1t[p, j*N + n] = S_W1*W_in[n, v0+128j+p]
    w1t = nc.dram_tensor("w1t", [128, NJ * N], F8, kind="ExternalInput")
    # w2p: PE half, w2p[n, 128b+p] = S_W2*W_out[v0+125p+b, n], n < NPE
    w2p = nc.dram_tensor("w2p", [NPE, PEB * 128], F8, kind="ExternalInput")
    # w2p44: the PE-half blocks' n-tail, v-on-partitions (bf16, unscaled):
    # w2p44[p, b*NTL + k] = W_out[v0+125p+b, NPE+k]
    w2p44 = nc.dram_tensor("w2p44", [128, PEB * NTL], BF16, kind="ExternalInput")
    # w2r: DVE half, w2r[p, bb*N+n] = W_out[v0+125p+PEB+bb, n], bf16
    w2r = nc.dram_tensor("w2r", [128, DVB * N], BF16, kind="ExternalInput")
    y_out = nc.dram_tensor("y", [128, NB], F32, kind="ExternalOutput")

    with tile.TileContext(nc) as tc:
        with (
            tc.tile_pool(name="const", bufs=1) as cpool,
            tc.tile_pool(name="scr", bufs=2) as spool,
            tc.tile_pool(name="psum", bufs=1, space="PSUM") as ppool,
            tc.tile_pool(name="dram", bufs=1, space="DRAM") as dpool,
        ):
            # ---- constants / inputs staged early ----
            ctx_sb = cpool.tile([128, NJ * C], F8, tag="ctx")
            nc.gpsimd.dma_start(ctx_sb[:, :], ctxp[:, :])

            ones10 = cpool.tile([C, 1], F32, tag="ones10")
            nc.vector.memset(ones10[:, :], 1.0)
            ones8 = cpool.tile([W, 1], BF16, tag="ones8")
            nc.vector.memset(ones8[:, :], 1.0)
            onesrow = cpool.tile([1, 128], BF16, tag="onesrow")
            nc.vector.memset(onesrow[:, :], 1.0)
            ones1f = cpool.tile([1, 128], F32, tag="ones1f")
            nc.vector.memset(ones1f[:, :], 1.0)
            ones128 = cpool.tile([128, 1], F32, tag="ones128")
            nc.vector.memset(ones128[:, :], 1.0)
            ident1 = cpool.tile([1, 1], F32, tag="ident1")
            nc.vector.memset(ident1[:, :], 1.0)

            # ---- w1 stream first (both HWDGE rings) ----
            w1_groups = [3, 7, 15] + [20] * 5  # sums to 125
            w1_sb = []
            j0 = 0
            for g, nj in enumerate(w1_groups):
                t = cpool.tile([128, nj * N], F8, tag=f"w1_{g}")
                ring = nc.sync if g % 2 == 0 else nc.scalar
                ring.dma_start(t[:, :], w1t[:, j0 * N:(j0 + nj) * N])
                w1_sb.append((t, j0, nj))
                j0 += nj

            # w2 streams right behind w1, alternating rings
            w2i = 0
            w2_sb = {}
            for g in range(NG2):
                b0 = g * W2G
                nb = min(W2G, PEB - b0)
                for i3, (off, kk) in enumerate(NCH_PE):
                    t = cpool.tile([kk, nb * 128], F8, tag=f"w2_{i3}_{g}")
                    ring = nc.sync if w2i % 2 == 0 else nc.scalar
                    w2i += 1
                    ring.dma_start(
                        t[:, :], w2p[off:off + kk, b0 * 128:(b0 + nb) * 128]
                    )
                    w2_sb[(i3, g)] = t
            w2p44_sb = cpool.tile([128, PEB * NTL], BF16, tag="w2p44")
            nc.sync.dma_start(w2p44_sb[:, :], w2p44[:, :])
            w2i += 1
            w2r_sb = []
            for g in range(NGR):
                bb0 = g * DVG
                nb = min(DVG, DVB - bb0)
                t = cpool.tile([128, nb * N], BF16, tag=f"w2r_{g}")
                ring = nc.sync if w2i % 2 == 0 else nc.scalar
                w2i += 1
                ring.dma_start(t[:, :], w2r[:, bb0 * N:(bb0 + nb) * N])
                w2r_sb.append((t, bb0, nb))

            # ---- GEMM1: psum_h[c, n] += ctx_chunk^T x w1 tile (fp8) ----
            psum_h = ppool.tile([C, N], F32, tag="ph")
            for t, j0g, nj in w1_sb:
                for jj in range(nj):
                    j = j0g + jj
                    nc.tensor.matmul(
                        psum_h[:, :],
                        ctx_sb[:, j * C:(j + 1) * C],
                        t[:, jj * N:(jj + 1) * N],
                        start=(j == 0),
                        stop=(j == NJ - 1),
                    )

            # ---- local C-reduce -> [1, 300] bf16 AllGather payload ----
            h10 = cpool.tile([C, N], F32, tag="h10")
            nc.vector.tensor_copy(h10[:, :], psum_h[:, :])
            psum_hl = ppool.tile([1, N], F32, tag="phl")
            nc.tensor.matmul(psum_hl[:, :], ones10[:, :], h10[:, :])
            h_loc = cpool.tile([1, N], BF16, tag="hloc")
            nc.vector.tensor_copy(h_loc[:, :], psum_hl[:, :])

            # cc_in write rides gpsimd: the HWDGE rings are still busy
            # streaming w2 at this point and would head-of-line block it
            cc_in = dpool.tile([1, N], BF16, tag="cc_in")
            cc_out = dpool.tile([W, N], BF16, tag="cc_out")
            nc.gpsimd.dma_start(cc_in[:, :], h_loc[:, :])
            nc.gpsimd.collective_compute(
                "AllGather",
                mybir.AluOpType.bypass,
                replica_groups=[list(range(W))],
                ins=[cc_in.opt()],
                outs=[cc_out.opt()],
            )
            hall = cpool.tile([W, N], BF16, tag="hall")
            nc.sync.dma_start(hall[:, :], cc_out[:, :])

            # ---- rank-sum; psum_hf = S_W1*C*hidden (f32, exact) ----
            psum_hf = ppool.tile([1, N], F32, tag="phf")
            nc.tensor.matmul(psum_hf[:, :], ones8[:, :], hall[:, :])
            # h_f32 = S_H * hidden (for the PE-half fp8 stationary)
            h_f32 = cpool.tile([1, N], F32, tag="hf32")
            nc.vector.tensor_scalar_mul(h_f32[:, :], psum_hf[:, :], S_H / (S_W1 * C))
            # h_bf = hidden (bf16, for the DVE half)
            h_bf = cpool.tile([1, N], BF16, tag="hbf")
            nc.vector.tensor_scalar_mul(h_bf[:, :], psum_hf[:, :], 1.0 / (S_W1 * C))

            # n-on-partitions fp8 copy for the PE half (3 PE transposes)
            psum_t = ppool.tile([128, 3], F32, tag="pt")
            for i3, (off, kk) in enumerate(NCH):
                nc.tensor.transpose(
                    psum_t[0:kk, i3:i3 + 1], h_f32[:, off:off + kk], ident1[:, :]
                )
            h_nt = cpool.tile([128, 3], F8, tag="hnt")
            nc.vector.tensor_copy(h_nt[:, 0:2], psum_t[:, 0:2])
            nc.vector.tensor_copy(h_nt[0:44, 2:3], psum_t[0:44, 2:3])

            # partition-broadcast hidden for the DVE half (rank-1 matmul)
            psum_r = ppool.tile([128, N], F32, tag="pr")
            nc.tensor.matmul(psum_r[:, :], onesrow[:, :], h_bf[:, :])
            h_rep = cpool.tile([128, N], BF16, tag="hrep")
            nc.vector.tensor_copy(h_rep[:, :], psum_r[:, :])

            # ---- GEMM2 PE half: logits[p, b]*(S_W2*S_H) over n<NPE ----
            psum_l = ppool.tile([128, PEB], F32, tag="pl")
            for b in range(PEB):
                g, bb = divmod(b, W2G)
                for i3, (off, kk) in enumerate(NCH_PE):
                    nc.tensor.matmul(
                        psum_l[:, b:b + 1],
                        w2_sb[(i3, g)][:, bb * 128:(bb + 1) * 128],
                        h_nt[0:kk, i3:i3 + 1],
                        start=(i3 == 0),
                        stop=(i3 == 1),
                    )

            # ---- PE-half n-tail on DVE: lg44[p, b] = sum_k w2p44*h ----
            lg44 = cpool.tile([128, PEB], F32, tag="lg44")
            h44 = h_rep[:, NPE:N].rearrange("p (x n) -> p x n", x=1)
            for g in range(NG2):
                b0 = g * W2G
                nb = min(W2G, PEB - b0)
                s44 = spool.tile([128, nb * NTL], BF16, tag="s44")
                sv44 = s44[:, :].rearrange("p (b n) -> p b n", b=nb)
                nc.vector.tensor_mul(
                    sv44,
                    w2p44_sb[:, b0 * NTL:(b0 + nb) * NTL]
                    .rearrange("p (b n) -> p b n", b=nb),
                    h44.broadcast_to([128, nb, NTL]),
                )
                a44 = spool.tile([128, nb * 22], BF16, tag="a44")
                av44 = a44[:, :].rearrange("p (b n) -> p b n", b=nb)
                nc.vector.tensor_add(av44, sv44[:, :, 0:22], sv44[:, :, 22:44])
                nc.vector.tensor_reduce(
                    lg44[:, b0:b0 + nb], av44,
                    mybir.AxisListType.X, mybir.AluOpType.add,
                )

            # ---- GEMM2 DVE half (bf16): grouped tensor_mul + 2-level
            #      tree-add (300->150->75) + one small segmented reduce.
            #      Per-block reduces cost ~290ns of fixed overhead each;
            #      the tree does a whole group in 3 DVE ops. ----
            lg_dve = cpool.tile([128, DVB], F32, tag="lgd")
            for t, bb0, nb in w2r_sb:
                scr = spool.tile([128, nb * N], BF16, tag="ttr_scr")
                t1 = spool.tile([128, nb * 150], BF16, tag="ttr_t1")
                t2 = spool.tile([128, nb * 75], BF16, tag="ttr_t2")
                h_b = h_rep[:, :].rearrange("p (x n) -> p x n", x=1)
                sv = scr[:, :].rearrange("p (b n) -> p b n", b=nb)
                nc.vector.tensor_mul(
                    sv,
                    t[:, 0:nb * N].rearrange("p (b n) -> p b n", b=nb),
                    h_b.broadcast_to([128, nb, N]),
                )
                t1v = t1[:, :].rearrange("p (b n) -> p b n", b=nb)
                nc.vector.tensor_add(t1v, sv[:, :, 0:150], sv[:, :, 150:300])
                t2v = t2[:, :].rearrange("p (b n) -> p b n", b=nb)
                nc.vector.tensor_add(t2v, t1v[:, :, 0:75], t1v[:, :, 75:150])
                nc.vector.tensor_reduce(
                    lg_dve[:, bb0:bb0 + nb], t2v,
                    mybir.AxisListType.X, mybir.AluOpType.add,
                )

            # ---- softmax (exp scales undo the fp8 pre-scales exactly) ----
            # combine the PE logits (x S_W2*S_H) with the n-tail (x1)
            lgpe = cpool.tile([128, PEB], F32, tag="lgpe")
            nc.vector.scalar_tensor_tensor(
                lgpe[:, :],
                psum_l[:, :],
                1.0 / (S_W2 * S_H),
                lg44[:, :],
                mybir.AluOpType.mult,
                mybir.AluOpType.add,
            )
            e_sb = cpool.tile([128, NB], F32, tag="esb")
            esum2 = cpool.tile([128, 2], F32, tag="esum2")
            nc.scalar.activation(
                e_sb[:, 0:PEB],
                lgpe[:, :],
                mybir.ActivationFunctionType.Exp,
                accum_out=esum2[:, 0:1],
            )
            nc.scalar.activation(
                e_sb[:, PEB:NB],
                lg_dve[:, :],
                mybir.ActivationFunctionType.Exp,
                accum_out=esum2[:, 1:2],
            )
            # reuse the (long-idle) psum_hl bank for the denominator sums
            nc.tensor.matmul(psum_hl[:, 0:2], ones128[:, :], esum2[:, :])
            ls = cpool.tile([1, 1], F32, tag="ls")
            nc.vector.tensor_reduce(
                ls[:, :], psum_hl[:, 0:2], mybir.AxisListType.X, mybir.AluOpType.add
            )

            cc2_in = dpool.tile([1, 1], F32, tag="cc2_in")
            cc2_out = dpool.tile([1, W], F32, tag="cc2_out")
            nc.sync.dma_start(cc2_in[:, :], ls[:, :])
            nc.gpsimd.collective_compute(
                "AllGather",
                mybir.AluOpType.bypass,
                replica_groups=[list(range(W))],
                ins=[cc2_in.opt()],
                outs=[cc2_out.opt()],
            )
            zs = cpool.tile([1, W], F32, tag="zs")
            nc.sync.dma_start(zs[:, :], cc2_out[:, :])
            zt = cpool.tile([1, 1], F32, tag="zt")
            nc.vector.tensor_reduce(
                zt[:, :], zs[:, :], mybir.AxisListType.X, mybir.AluOpType.add
            )
            # reciprocal broadcast to all partitions via rank-1 ones-matmul
            # (reusing the psum_t bank)
            nc.tensor.matmul(psum_t[:, 0:1], ones1f[:, :], zt[:, :])
            rb = cpool.tile([128, 1], F32, tag="rb")
            nc.vector.reciprocal(rb[:, :], psum_t[:, 0:1])

            y_sb = cpool.tile([128, NB], F32, tag="ysb")
            nc.vector.tensor_scalar_mul(y_sb[:, :], e_sb[:, :], rb[:, :])
            nc.sync.dma_start(y_out[:, :], y_sb[:, :])

    _split_multi_waits(nc)
    return nc


_NC_CACHE = None


def _get_nc():
    global _NC_CACHE
    if _NC_CACHE is None:
        _NC_CACHE = build_kernel()
    return _NC_CACHE


def _prep_inputs(context_words, W_in, W_out):
    """Host-side shard + layout prep (pure data movement + dtype cast)."""
    in_maps = []
    W_in_s = (np.asarray(W_in, dtype=np.float32) * S_W1).astype(NP_F8)
    W_out_s8 = (np.asarray(W_out, dtype=np.float32) * S_W2).astype(NP_F8)
    W_out_b = np.asarray(W_out, dtype=NP_BF16)
    ctx_f8 = np.asarray(context_words, dtype=NP_F8)
    for r in range(W):
        v0 = r * VL
        ctx_s = ctx_f8[:, v0:v0 + VL]
        # ctxp[p, j*C + c] = ctx[c, 128j + p]
        ctxp = np.ascontiguousarray(
            ctx_s.reshape(C, NJ, 128).transpose(2, 1, 0).reshape(128, NJ * C)
        )
        # w1t[p, j*N + n] = S_W1*W_in[n, v0 + 128j + p]  (partition-major pack)
        w1t = np.ascontiguousarray(
            W_in_s[:, v0:v0 + VL].T
            .reshape(NJ, 128, N).transpose(1, 0, 2).reshape(128, NJ * N)
        )
        # PE half: w2p[n, 128b + p] = S_W2*W_out[v0 + 125p + b, n], n < NPE
        ws8 = W_out_s8[v0:v0 + VL, :].reshape(128, NB, N)
        w2p = np.ascontiguousarray(
            ws8[:, :PEB, :NPE].transpose(2, 1, 0).reshape(NPE, PEB * 128)
        )
        wsb = W_out_b[v0:v0 + VL, :].reshape(128, NB, N)
        # PE-half n-tail (bf16, v-on-partitions, unscaled)
        w2p44 = np.ascontiguousarray(
            wsb[:, :PEB, NPE:].reshape(128, PEB * NTL)
        )
        # DVE half: w2r[p, bb*N + n] = W_out[v0 + 125p + PEB+bb, n] (bf16)
        w2r = np.ascontiguousarray(wsb[:, PEB:, :].reshape(128, DVB * N))
        in_maps.append(
            {"ctxp": ctxp, "w1t": w1t, "w2p": w2p, "w2p44": w2p44, "w2r": w2r}
        )
    return in_maps


def kernel(context_words, W_in, W_out):
    nc = _get_nc()
    in_maps = _prep_inputs(context_words, W_in, W_out)
    res = run_bass_kernel_spmd(nc, in_maps, list(range(W)))
    # y[p, b] on core r = prob[r*VL + 125*p + b]
    return np.concatenate(
        [np.asarray(res.results[r]["y"], dtype=np.float32).reshape(VL) for r in range(W)]
    )


# revision 40
# speedup vs baseline: 1.0127x; 1.0127x over previous
"""CBOW (nn_CBOW_88991722373900) Trainium2 kernel.

Full-input contract: kernel(context_words[10,128000] f32, W_in[300,128000] f32,
W_out[128000,300] f32) -> softmax probabilities [128000] f32.

Strategy (8-way tensor/model parallel over the vocab dim V):
  - shard V into 8 chunks of 16000; each core holds its slice of both weight
    matrices. W_in/ctx/PE-half W_out in fp8-e4m3 (host pre-scales W_in x128,
    W_out x16, hidden x8, compensated exactly in the exp scale); the
    DVE-half W_out stays bf16 (DVE has no fp8 packing).
  - DMA order: w1 groups first on both HWDGE rings, then w2, so GEMM1
    finishes and triggers the hidden AllGather before the runtime's CC
    barrier completes (the barrier starts ~21us into execution on its own
    and gates the first collective; GEMM1 speed is off the critical path).
  - GEMM1: partial hidden[10,300] = ctx chunks (stationary) x w1 tiles
    (moving), accumulated in PSUM over 125 v-chunks (fp8 matmuls).
  - C-reduce (ones-matmul) -> AllGather(600B bf16) -> rank-sum.
  - GEMM2 split across two engines (post-AllGather critical path):
      PE half  (v-blocks 0..PEB-1): W_out col-blocks stationary (fp8, FWL),
               hidden col (fp8 x8) moving -> PSUM [128, PEB], logits x128
      DVE half: fused tensor_tensor_reduce of bf16 w2r against the
               partition-broadcast bf16 hidden, one op per v-block
    v mapped so partition p holds contiguous v = 125*p + b
  - softmax: exp on ScalarE with scale folding the fp8 pre-scales,
    AllGather(4B) of the local denominators, then a rank-1 ones-matmul
    broadcasts the reciprocal to all partitions (no broadcast-read DMA).
  - all small/control DMAs ride the HWDGE rings (0.6us fixed cost vs
    ~2us on the SWDGE/gpsimd path).
"""

import numpy as np
import ml_dtypes

import concourse.bass as bass
import concourse.mybir as mybir
from concourse import tile
from concourse.bass_utils import run_bass_kernel_spmd
from concourse.vector_clock import ScopedClock, VectorClock

V = 128000
N = 300
C = 10
W = 8              # cores
VL = V // W        # 16000 vocab per core
NJ = VL // 128     # 125 v-chunks for GEMM1
NB = VL // 128     # 125 v-blocks for GEMM2
NCH = [(0, 128), (128, 128), (256, 44)]  # n-chunks
PEB = 70           # v-blocks on the PE half of GEMM2
DVB = NB - PEB     # v-blocks on the DVE half
W2G = 18           # PE-half v-blocks per w2 SBUF tile group
NG2 = (PEB + W2G - 1) // W2G
DVG = 14           # DVE-half v-blocks per w2r SBUF tile group
NGR = (DVB + DVG - 1) // DVG
# PE half covers n in [0, 256) (2 LDW+MM pairs per block instead of 3);
# the n in [256, 300) tail of those blocks runs on the DVE (w2p44).
NPE = 256
NTL = N - NPE      # 44
NCH_PE = [(0, 128), (128, 128)]

# host-side power-of-2 pre-scales (compensated exactly on-chip)
S_W1 = 128.0       # W_in scale into fp8 range
S_W2 = 16.0        # W_out scale into fp8 range (PE half only)
S_H = 8.0          # hidden scale into fp8 range (PE half)

BF16 = mybir.dt.bfloat16
F8 = mybir.dt.float8e4
F32 = mybir.dt.float32
F32R = mybir.dt.float32r
NP_BF16 = ml_dtypes.bfloat16
NP_F8 = ml_dtypes.float8_e4m3


def _patched_drain_and_barrier(self, tick_clock, wait_clock):
    """Tail-drain waits split into 1-wait NOPs: this walrus build's CTRL
    instructions only encode a single sync wait."""
    vc = tick_clock.global_clock
    procs = [(p, vc[p]) for p in range(len(vc)) if vc[p] > 0]
    for i, (p, t) in enumerate(procs):
        pvc = VectorClock([0] * len(vc))
        pvc.require_at_least(p, t)
        nop_inst = self.nc.sync.nop(nofuse=True, hint=f"tail_wait_{i}")
        wait_clock.add_sem_waits(nop_inst.ins, ScopedClock({None: pvc}))
    self.nc.sync.drain()
    self.nc.all_engine_barrier(sem_only=True)
    assert self.sems is not None
    popped = self.nc._tile_sem_poison_stack.pop()
    assert popped is self._sem_poison
    self.nc.clear_and_free_semaphores(list(self.sems.allocated().values()))
    self.nc.all_engine_barrier(sem_only=True)


tile.TileContext._drain_and_barrier = _patched_drain_and_barrier


def _split_multi_waits(nc):
    """This walrus build encodes at most ONE sync wait per instruction. Hoist
    excess waits onto same-engine NoOps inserted immediately before."""
    import bass_rust

    ctr = [0]

    def make_nop(engine, wait):
        ctr[0] += 1
        nop = mybir.InstNoOp(name=f"I-wsplit{ctr[0]}", engine=engine)
        nop.bass_nofuse = True
        nop.sync_info = bass_rust.SyncInfo(on_wait=[wait], on_update=[])
        nc.register_instruction(nop, overwrite=True)
        return nop

    for bb in nc.main_func.blocks:
        out = []
        for ins in bb.instructions:
            si = ins.sync_info
            if si is not None and si.on_wait and len(si.on_wait) > 1:
                waits = list(si.on_wait)
                for w in waits[:-1]:
                    out.append(make_nop(ins.engine, w))
                ins.sync_info = bass_rust.SyncInfo(
                    on_wait=[waits[-1]], on_update=list(si.on_update)
                )
            out.append(ins)
        bb.instructions = out


def build_kernel():
    nc = bass.Bass()

    ctxp = nc.dram_tensor("ctxp", [128, NJ * C], F8, kind="ExternalInput")
    # w1t packed partition-major on host: w1t[p, j*N + n] = S_W1*W_in[n, v0+128j+p]
    w1t = nc.dram_tensor("w1t", [128, NJ * N], F8, kind="ExternalInput")
    # w2p: PE half, w2p[n, 128b+p] = S_W2*W_out[v0+125p+b, n], n < NPE
    w2p = nc.dram_tensor("w2p", [NPE, PEB * 128], F8, kind="ExternalInput")
    # w2p44: the PE-half blocks' n-tail, v-on-partitions (bf16, unscaled):
    # w2p44[p, b*NTL + k] = W_out[v0+125p+b, NPE+k]
    w2p44 = nc.dram_tensor("w2p44", [128, PEB * NTL], BF16, kind="ExternalInput")
    # w2r: DVE half, w2r[p, bb*N+n] = W_out[v0+125p+PEB+bb, n], bf16
    w2r = nc.dram_tensor("w2r", [128, DVB * N], BF16, kind="ExternalInput")
    y_out = nc.dram_tensor("y", [128, NB], F32, kind="ExternalOutput")

    with tile.TileContext(nc) as tc:
        with (
            tc.tile_pool(name="const", bufs=1) as cpool,
            tc.tile_pool(name="scr", bufs=2) as spool,
            tc.tile_pool(name="psum", bufs=1, space="PSUM") as ppool,
            tc.tile_pool(name="dram", bufs=1, space="DRAM") as dpool,
        ):
            # ---- constants / inputs staged early ----
            ctx_sb = cpool.tile([128, NJ * C], F8, tag="ctx")
            nc.gpsimd.dma_start(ctx_sb[:, :], ctxp[:, :])

            ones10 = cpool.tile([C, 1], F32, tag="ones10")
            nc.vector.memset(ones10[:, :], 1.0)
            ones8 = cpool.tile([W, 1], BF16, tag="ones8")
            nc.vector.memset(ones8[:, :], 1.0)
            onesrow = cpool.tile([1, 128], BF16, tag="onesrow")
            nc.vector.memset(onesrow[:, :], 1.0)
            ones1f = cpool.tile([1, 128], F32, tag="ones1f")
            nc.vector.memset(ones1f[:, :], 1.0)
            ones128 = cpool.tile([128, 1], F32, tag="ones128")
            nc.vector.memset(ones128[:, :], 1.0)
            ident1 = cpool.tile([1, 1], F32, tag="ident1")
            nc.vector.memset(ident1[:, :], 1.0)

            # ---- w1 stream first (both HWDGE rings) ----
            w1_groups = [3, 7, 15] + [20] * 5  # sums to 125
            w1_sb = []
            j0 = 0
            for g, nj in enumerate(w1_groups):
                t = cpool.tile([128, nj * N], F8, tag=f"w1_{g}")
                ring = nc.sync if g % 2 == 0 else nc.scalar
                ring.dma_start(t[:, :], w1t[:, j0 * N:(j0 + nj) * N])
                w1_sb.append((t, j0, nj))
                j0 += nj

            # w2 streams right behind w1, alternating rings
            w2i = 0
            w2_sb = {}
            for g in range(NG2):
                b0 = g * W2G
                nb = min(W2G, PEB - b0)
                for i3, (off, kk) in enumerate(NCH_PE):
                    t = cpool.tile([kk, nb * 128], F8, tag=f"w2_{i3}_{g}")
                    ring = nc.sync if w2i % 2 == 0 else nc.scalar
                    w2i += 1
                    ring.dma_start(
                        t[:, :], w2p[off:off + kk, b0 * 128:(b0 + nb) * 128]
                    )
                    w2_sb[(i3, g)] = t
            w2p44_sb = cpool.tile([128, PEB * NTL], BF16, tag="w2p44")
            nc.sync.dma_start(w2p44_sb[:, :], w2p44[:, :])
            w2i += 1
            w2r_sb = []
            for g in range(NGR):
                bb0 = g * DVG
                nb = min(DVG, DVB - bb0)
                t = cpool.tile([128, nb * N], BF16, tag=f"w2r_{g}")
                ring = nc.sync if w2i % 2 == 0 else nc.scalar
                w2i += 1
                ring.dma_start(t[:, :], w2r[:, bb0 * N:(bb0 + nb) * N])
                w2r_sb.append((t, bb0, nb))

            # ---- GEMM1: psum_h[c, n] += ctx_chunk^T x w1 tile (fp8) ----
            psum_h = ppool.tile([C, N], F32, tag="ph")
            for t, j0g, nj in w1_sb:
                for jj in range(nj):
                    j = j0g + jj
                    nc.tensor.matmul(
                        psum_h[:, :],
                        ctx_sb[:, j * C:(j + 1) * C],
                        t[:, jj * N:(jj + 1) * N],
                        start=(j == 0),
                        stop=(j == NJ - 1),
                    )

            # ---- local C-reduce -> [1, 300] bf16 AllGather payload ----
            h10 = cpool.tile([C, N], F32, tag="h10")
            nc.vector.tensor_copy(h10[:, :], psum_h[:, :])
            psum_hl = ppool.tile([1, N], F32, tag="phl")
            nc.tensor.matmul(psum_hl[:, :], ones10[:, :], h10[:, :])
            h_loc = cpool.tile([1, N], BF16, tag="hloc")
            nc.vector.tensor_copy(h_loc[:, :], psum_hl[:, :])

            # cc_in write rides gpsimd: the HWDGE rings are still busy
            # streaming w2 at this point and would head-of-line block it
            cc_in = dpool.tile([1, N], BF16, tag="cc_in")
            cc_out = dpool.tile([W, N], BF16, tag="cc_out")
            nc.gpsimd.dma_start(cc_in[:, :], h_loc[:, :])
            nc.gpsimd.collective_compute(
                "AllGather",
                mybir.AluOpType.bypass,
                replica_groups=[list(range(W))],
                ins=[cc_in.opt()],
                outs=[cc_out.opt()],
            )
            hall = cpool.tile([W, N], BF16, tag="hall")
            nc.sync.dma_start(hall[:, :], cc_out[:, :])

            # ---- rank-sum; psum_hf = S_W1*C*hidden (f32, exact) ----
            psum_hf = ppool.tile([1, N], F32, tag="phf")
            nc.tensor.matmul(psum_hf[:, :], ones8[:, :], hall[:, :])
            # h_f32 = S_H * hidden (for the PE-half fp8 stationary)
            h_f32 = cpool.tile([1, N], F32, tag="hf32")
            nc.vector.tensor_scalar_mul(h_f32[:, :], psum_hf[:, :], S_H / (S_W1 * C))
            # h_bf = hidden (bf16, for the DVE half)
            h_bf = cpool.tile([1, N], BF16, tag="hbf")
            nc.vector.tensor_scalar_mul(h_bf[:, :], psum_hf[:, :], 1.0 / (S_W1 * C))

            # n-on-partitions fp8 copy for the PE half (3 PE transposes)
            psum_t = ppool.tile([128, 3], F32, tag="pt")
            for i3, (off, kk) in enumerate(NCH):
                nc.tensor.transpose(
                    psum_t[0:kk, i3:i3 + 1], h_f32[:, off:off + kk], ident1[:, :]
                )
            h_nt = cpool.tile([128, 3], F8, tag="hnt")
            nc.vector.tensor_copy(h_nt[:, 0:2], psum_t[:, 0:2])
            nc.vector.tensor_copy(h_nt[0:44, 2:3], psum_t[0:44, 2:3])

            # partition-broadcast hidden for the DVE half (rank-1 matmul)
            psum_r = ppool.tile([128, N], F32, tag="pr")
            nc.tensor.matmul(psum_r[:, :], onesrow[:, :], h_bf[:, :])
            h_rep = cpool.tile([128, N], BF16, tag="hrep")
            nc.vector.tensor_copy(h_rep[:, :], psum_r[:, :])

            # ---- GEMM2 PE half: logits[p, b]*(S_W2*S_H) over n<NPE ----
            psum_l = ppool.tile([128, PEB], F32, tag="pl")
            for b in range(PEB):
                g, bb = divmod(b, W2G)
                for i3, (off, kk) in enumerate(NCH_PE):
                    nc.tensor.matmul(
                        psum_l[:, b:b + 1],
                        w2_sb[(i3, g)][:, bb * 128:(bb + 1) * 128],
                        h_nt[0:kk, i3:i3 + 1],
                        start=(i3 == 0),
                        stop=(i3 == 1),
                    )

            # ---- PE-half n-tail on DVE: lg44[p, b] = sum_k w2p44*h ----
            lg44 = cpool.tile([128, PEB], F32, tag="lg44")
            h44 = h_rep[:, NPE:N].rearrange("p (x n) -> p x n", x=1)
            for g in range(NG2):
                b0 = g * W2G
                nb = min(W2G, PEB - b0)
                s44 = spool.tile([128, nb * NTL], BF16, tag="s44")
                sv44 = s44[:, :].rearrange("p (b n) -> p b n", b=nb)
                nc.vector.tensor_mul(
                    sv44,
                    w2p44_sb[:, b0 * NTL:(b0 + nb) * NTL]
                    .rearrange("p (b n) -> p b n", b=nb),
                    h44.broadcast_to([128, nb, NTL]),
                )
                a44 = spool.tile([128, nb * 22], BF16, tag="a44")
                av44 = a44[:, :].rearrange("p (b n) -> p b n", b=nb)
                nc.vector.tensor_add(av44, sv44[:, :, 0:22], sv44[:, :, 22:44])
                nc.vector.tensor_reduce(
                    lg44[:, b0:b0 + nb], av44,
                    mybir.AxisListType.X, mybir.AluOpType.add,
                )

            # ---- GEMM2 DVE half (bf16): grouped tensor_mul + 2-level
            #      tree-add (300->150->75) + one small segmented reduce.
            #      Per-block reduces cost ~290ns of fixed overhead each;
            #      the tree does a whole group in 3 DVE ops. ----
            lg_dve = cpool.tile([128, DVB], F32, tag="lgd")
            for t, bb0, nb in w2r_sb:
                scr = spool.tile([128, nb * N], BF16, tag="ttr_scr")
                t1 = spool.tile([128, nb * 150], BF16, tag="ttr_t1")
                t2 = spool.tile([128, nb * 75], BF16, tag="ttr_t2")
                h_b = h_rep[:, :].rearrange("p (x n) -> p x n", x=1)
                sv = scr[:, :].rearrange("p (b n) -> p b n", b=nb)
                nc.vector.tensor_mul(
                    sv,
                    t[:, 0:nb * N].rearrange("p (b n) -> p b n", b=nb),
                    h_b.broadcast_to([128, nb, N]),
                )
                t1v = t1[:, :].rearrange("p (b n) -> p b n", b=nb)
                nc.vector.tensor_add(t1v, sv[:, :, 0:150], sv[:, :, 150:300])
                t2v = t2[:, :].rearrange("p (b n) -> p b n", b=nb)
                nc.vector.tensor_add(t2v, t1v[:, :, 0:75], t1v[:, :, 75:150])
                nc.vector.tensor_reduce(
                    lg_dve[:, bb0:bb0 + nb], t2v,
                    mybir.AxisListType.X, mybir.AluOpType.add,
                )

            # ---- softmax (exp scales undo the fp8 pre-scales exactly) ----
            # combine the PE logits (x S_W2*S_H) with the n-tail (x1)
            lgpe = cpool.tile([128, PEB], F32, tag="lgpe")
            nc.vector.scalar_tensor_tensor(
                lgpe[:, :],
                psum_l[:, :],
                1.0 / (S_W2 * S_H),
                lg44[:, :],
                mybir.AluOpType.mult,
                mybir.AluOpType.add,
            )
            e_sb = cpool.tile([128, NB], F32, tag="esb")
            esum2 = cpool.tile([128, 2], F32, tag="esum2")
            nc.scalar.activation(
                e_sb[:, 0:PEB],
                lgpe[:, :],
                mybir.ActivationFunctionType.Exp,
                accum_out=esum2[:, 0:1],
            )
            nc.scalar.activation(
                e_sb[:, PEB:NB],
                lg_dve[:, :],
                mybir.ActivationFunctionType.Exp,
                accum_out=esum2[:, 1:2],
            )
            # reuse the (long-idle) psum_hl bank for the denominator sums
            nc.tensor.matmul(psum_hl[:, 0:2], ones128[:, :], esum2[:, :])
            ls = cpool.tile([1, 1], F32, tag="ls")
            nc.vector.tensor_reduce(
                ls[:, :], psum_hl[:, 0:2], mybir.AxisListType.X, mybir.AluOpType.add
            )

            cc2_in = dpool.tile([1, 1], F32, tag="cc2_in")
            cc2_out = dpool.tile([1, W], F32, tag="cc2_out")
            nc.sync.dma_start(cc2_in[:, :], ls[:, :])
            nc.gpsimd.collective_compute(
                "AllGather",
                mybir.AluOpType.bypass,
                replica_groups=[list(range(W))],
                ins=[cc2_in.opt()],
                outs=[cc2_out.opt()],
            )
            zs = cpool.tile([1, W], F32, tag="zs")
            nc.sync.dma_start(zs[:, :], cc2_out[:, :])
            zt = cpool.tile([1, 1], F32, tag="zt")
            nc.vector.tensor_reduce(
                zt[:, :], zs[:, :], mybir.AxisListType.X, mybir.AluOpType.add
            )
            # reciprocal broadcast to all partitions via rank-1 ones-matmul
            # (reusing the psum_t bank)
            nc.tensor.matmul(psum_t[:, 0:1], ones1f[:, :], zt[:, :])
            rb = cpool.tile([128, 1], F32, tag="rb")
            nc.vector.reciprocal(rb[:, :], psum_t[:, 0:1])

            y_sb = cpool.tile([128, NB], F32, tag="ysb")
            nc.vector.tensor_scalar_mul(y_sb[:, :], e_sb[:, :], rb[:, :])
            nc.sync.dma_start(y_out[:, :], y_sb[:, :])

    _split_multi_waits(nc)
    return nc


_NC_CACHE = None


def _get_nc():
    global _NC_CACHE
    if _NC_CACHE is None:
        _NC_CACHE = build_kernel()
    return _NC_CACHE


def _prep_inputs(context_words, W_in, W_out):
    """Host-side shard + layout prep (pure data movement + dtype cast)."""
    in_maps = []
    W_in_s = (np.asarray(W_in, dtype=np.float32) * S_W1).astype(NP_F8)
    W_out_s8 = (np.asarray(W_out, dtype=np.float32) * S_W2).astype(NP_F8)
    W_out_b = np.asarray(W_out, dtype=NP_BF16)
    ctx_f8 = np.asarray(context_words, dtype=NP_F8)
    for r in range(W):
        v0 = r * VL
        ctx_s = ctx_f8[:, v0:v0 + VL]
        # ctxp[p, j*C + c] = ctx[c, 128j + p]
        ctxp = np.ascontiguousarray(
            ctx_s.reshape(C, NJ, 128).transpose(2, 1, 0).reshape(128, NJ * C)
        )
        # w1t[p, j*N + n] = S_W1*W_in[n, v0 + 128j + p]  (partition-major pack)
        w1t = np.ascontiguousarray(
            W_in_s[:, v0:v0 + VL].T
            .reshape(NJ, 128, N).transpose(1, 0, 2).reshape(128, NJ * N)
        )
        # PE half: w2p[n, 128b + p] = S_W2*W_out[v0 + 125p + b, n], n < NPE
        ws8 = W_out_s8[v0:v0 + VL, :].reshape(128, NB, N)
        w2p = np.ascontiguousarray(
            ws8[:, :PEB, :NPE].transpose(2, 1, 0).reshape(NPE, PEB * 128)
        )
        wsb = W_out_b[v0:v0 + VL, :].reshape(128, NB, N)
        # PE-half n-tail (bf16, v-on-partitions, unscaled)
        w2p44 = np.ascontiguousarray(
            wsb[:, :PEB, NPE:].reshape(128, PEB * NTL)
        )
        # DVE half: w2r[p, bb*N + n] = W_out[v0 + 125p + PEB+bb, n] (bf16)
        w2r = np.ascontiguousarray(wsb[:, PEB:, :].reshape(128, DVB * N))
        in_maps.append(
            {"ctxp": ctxp, "w1t": w1t, "w2p": w2p, "w2p44": w2p44, "w2r": w2r}
        )
    return in_maps


def kernel(context_words, W_in, W_out):
    nc = _get_nc()
    in_maps = _prep_inputs(context_words, W_in, W_out)
    res = run_bass_kernel_spmd(nc, in_maps, list(range(W)))
    # y[p, b] on core r = prob[r*VL + 125*p + b]
    return np.concatenate(
        [np.asarray(res.results[r]["y"], dtype=np.float32).reshape(VL) for r in range(W)]
    )
